# revision 5
# baseline (speedup 1.0000x reference)
"""B3-spline undecimated wavelet transform (3 levels, reflect BC) on 8 trn2 cores.

Strategy
--------
Pure data parallel: 16 images -> 2 images per core.

Per level the separable 5-tap conv y = K_d @ Y @ K_d^T is computed as two
TensorEngine passes that each convolve along the *partition* axis and
transpose "for free":

    pass1:  AT = (K @ Y)^T      matmul(lhsT=Y_block, rhs=K^T_block)
    pass2:  Ynew = (K @ AT)^T   matmul(lhsT=AT_block, rhs=K^T_block)

K_d is banded (halfwidth 2d <= 8), so for each 128-row contraction block cb
only a narrow output window [cb*128-hw, cb*128+128+hw) is nonzero; each
window is issued as 1-2 matmuls (split at the 512-col PSUM bank boundary)
accumulating into a [128,1024] PSUM tile via the per-element has_written
bits.

HBM traffic is minimized: x is staged fp16 by the host (the conv weights
are dyadic rationals and exact in fp16, accumulation is fp32 in PSUM), and
all 4 output planes are stored fp16 and widened to fp32 on the host, so
per-core traffic is 4.2 MB in + 16.8 MB out (vs 8.4 + 33.6 for fp32).

The two images per core are interleaved pass-by-pass so one image's matmul
stream covers the other's PSUM-evacuation latency. PSUM evacuation (the
only PSUM readers are DVE and ACT) is split per-tile between the two, with
the wavelet subtraction w = Y_prev - Y running fp16 SBUF->SBUF (2x DVE
mode) and a few subs offloaded to GPSIMD.
"""

import sys

if "/opt/trn_rl_repo" not in sys.path:
    sys.path.insert(0, "/opt/trn_rl_repo")

import numpy as np

import concourse.bass as bass
import concourse.mybir as mybir
import concourse.tile as tile
from concourse import bacc
from concourse.bass_utils import run_bass_kernel_spmd

P = 128
L = 1024
NB = L // P            # 8 blocks per axis
BPC = 2                # images per core
NCORES = 8
LEVELS = (1, 2, 4)     # dilation per level
F32 = mybir.dt.float32
F16 = mybir.dt.float16
W5 = (1.0 / 16, 1.0 / 4, 3.0 / 8, 1.0 / 4, 1.0 / 16)

# Per-tile engine assignment (8 chars per level, one per mb tile):
#   'v' = DVE (vector), 's' = ACT (scalar), 'g' = GPSIMD (subs only).
P1_ENG = ("vsvsvsvs", "vsvsvsvs", "vsvsvsvs")   # pass1 A-evac (PSUM copy)
P2Y_ENG = ("svsvsvsv", "ssvsvssv", "svsvsvsv")  # pass2 Y-copy (PSUM copy)
SUB_ENG = ("gggggggg", "gggggggg", "gggggggg")  # w-sub (fp16 SBUF->SBUF)
L3_STORE_BLOCKS = 4  # h-blocks per last-level store DMA (1, 2, or 4)


def _conv_matrix(d: int) -> np.ndarray:
    """K such that (K @ x) == dilated reflect-padded 5-tap conv along axis 0."""
    eye = np.eye(L, dtype=np.float64)
    xp = np.pad(eye, ((2 * d, 2 * d), (0, 0)), mode="reflect")
    K = np.zeros((L, L), dtype=np.float64)
    for k in range(5):
        K += W5[k] * xp[k * d : k * d + L]
    return K.astype(np.float32)


def _const_arrays() -> dict[str, np.ndarray]:
    """fp16 K^T blocks per level: interior Toeplitz block + the two edge blocks."""
    consts = {}
    for li, d in enumerate(LEVELS):
        hw = 2 * d
        KT = _conv_matrix(d).T  # KT[i, n] = K[n, i]
        kint = KT[P : 2 * P, P - hw : 2 * P + hw]
        k0 = KT[0:P, 0 : P + hw]
        k7 = KT[7 * P : 8 * P, 7 * P - hw : 8 * P]
        for nm, a in ((f"kint{li}", kint), (f"k0{li}", k0), (f"k7{li}", k7)):
            a16 = np.ascontiguousarray(a, dtype=np.float16)
            assert np.array_equal(a16.astype(np.float32), a.astype(np.float32))
            consts[nm] = a16
    return consts


def _windows(li: int, cb: int):
    """Nonzero output-column segments for contraction block cb, split at the
    PSUM bank boundary. Returns [(c0, c1, const_name, rhs_col_offset)]."""
    hw = 2 * LEVELS[li]
    if cb == 0:
        c0, c1, nm, base = 0, P + hw, f"k0{li}", 0
    elif cb == NB - 1:
        c0, c1, nm, base = 7 * P - hw, L, f"k7{li}", 7 * P - hw
    else:
        c0, c1, nm, base = cb * P - hw, cb * P + P + hw, f"kint{li}", cb * P - hw
    segs = [(c0, 512), (512, c1)] if c0 < 512 < c1 else [(c0, c1)]
    return [(a, b, nm, a - base) for a, b in segs]


def _mm_list(li: int):
    """Ordered matmul segments for one PSUM tile with per-bank start/stop."""
    segs = []
    for cb in range(NB):
        for a, b, nm, off in _windows(li, cb):
            segs.append([cb, a, b, nm, off, False, False])
    first, last = {}, {}
    for i, s in enumerate(segs):
        bank = s[1] // 512
        first.setdefault(bank, i)
        last[bank] = i
    for i in first.values():
        segs[i][5] = True  # start: clears the bank's has_written bits
    for i in last.values():
        segs[i][6] = True  # stop: closes the accumulation group
    return [tuple(s) for s in segs]


def _conv_pass(nc, ksb, src_tiles, segs, pspool, consume):
    """One transposing conv pass: 8 src tiles [P, L] fp16 -> 8 PSUM tiles [P, L]."""
    for mb in range(NB):
        ps = pspool.tile([P, L], F32, tag="ps", name="ps")
        for cb, a, b, nm, off, st, sp in segs:
            nc.tensor.matmul(
                ps[:, a:b],
                src_tiles[cb][:, mb * P : (mb + 1) * P],
                ksb[nm][:, off : off + (b - a)],
                start=st,
                stop=sp,
            )
        consume(mb, ps)


def _build_nc(repeat: int = 1):
    consts = _const_arrays()
    nc = bacc.Bacc(
        "TRN2",
        target_bir_lowering=False,
        debug=False,
        num_devices=NCORES,
    )
    x_in = nc.dram_tensor("x", [BPC, L, L], F16, kind="ExternalInput")
    out = nc.dram_tensor("out", [BPC, 4, L, L], F16, kind="ExternalOutput")
    knames = list(consts)
    kwidths = [consts[nm].shape[1] for nm in knames]
    koffs = dict(zip(knames, np.cumsum([0] + kwidths[:-1]).tolist()))
    ktotal = int(sum(kwidths))
    kall = nc.dram_tensor("kall", [P, ktotal], F16, kind="ExternalInput")

    def eng(c):
        return {"v": nc.vector, "s": nc.scalar, "g": nc.gpsimd}[c]

    with tile.TileContext(nc) as tc:
        with (
            tc.tile_pool(name="consts", bufs=1) as cpool,
            tc.tile_pool(name="f16", bufs=3 * NB) as fpool,
            tc.tile_pool(name="at", bufs=2 * NB + 4) as apool,
            tc.tile_pool(name="wout", bufs=8) as wpool,
            tc.tile_pool(name="ps", bufs=4, space="PSUM") as pspool,
        ):
            kall_sb = cpool.tile([P, ktotal], F16, name="kall_sb")
            ksb = {
                nm: kall_sb[:, koffs[nm] : koffs[nm] + consts[nm].shape[1]]
                for nm in knames
            }

            kall_loaded = False

            def issue_loads():
                # fp16 x: per-block loads straight into matmul-input tiles,
                # split across the Pool and SP DGE queues. The first block's
                # DMA goes ahead of the (FIFO) const load.
                nonlocal kall_loaded
                cur = {}
                for img in range(BPC):
                    q = nc.gpsimd if img == 0 else nc.sync
                    cur[img] = []
                    for b in range(NB):
                        ct = fpool.tile([P, L], F16, tag="cur", name="cur")
                        q.dma_start(ct[:], x_in[img, b * P : (b + 1) * P])
                        cur[img].append(ct)
                        if not kall_loaded:
                            nc.scalar.dma_start(kall_sb[:], kall[:, :])
                            kall_loaded = True
                return cur

            pending_cur = issue_loads()
            for _rep in range(repeat):
                cur = pending_cur

                for li in range(len(LEVELS)):
                    segs = _mm_list(li)
                    last = li == len(LEVELS) - 1

                    # pass 1: AT = (K @ Y)^T, evacuated to fp16 per block.
                    # Image-interleaved: img1's matmuls cover img0's evacs.
                    at = {}
                    for img in range(BPC):
                        at[img] = [
                            apool.tile([P, L], F16, tag="at", name="at")
                            for _ in range(NB)
                        ]

                        def evac_at(mb, ps, at_i=at[img]):
                            e = P1_ENG[li][mb]
                            if e == "v":
                                nc.vector.tensor_copy(at_i[mb][:, :], ps[:, :])
                            else:
                                nc.scalar.copy(at_i[mb][:, :], ps[:, :])

                        _conv_pass(nc, ksb, cur[img], segs, pspool, evac_at)

                    # pass 2: Ynew = (K @ AT)^T. ACT/DVE copy Ynew fp16 (next
                    # level's input, or c3); the wavelet sub w = Y - Ynew runs
                    # fp16 SBUF->SBUF on DVE (2x mode) or GPSIMD. Output
                    # staged in half-image tiles so stores start early.
                    nxt = {}
                    for img in range(BPC):
                        w_halves = [
                            wpool.tile([P, NB // 2, L], F16, tag="w", name="w_sb")
                            for _ in range(2)
                        ]
                        c3_halves = (
                            [
                                wpool.tile(
                                    [P, NB // 2, L], F16, tag="w", name="c3_sb"
                                )
                                for _ in range(2)
                            ]
                            if last
                            else None
                        )
                        nxt[img] = (
                            None
                            if last
                            else [
                                fpool.tile([P, L], F16, tag="cur", name="nxt")
                                for _ in range(NB)
                            ]
                        )

                        def evac_y(
                            mb,
                            ps,
                            w=w_halves,
                            nxt_i=nxt[img],
                            c3=c3_halves,
                            carrier=cur[img],
                        ):
                            h, r = divmod(mb, NB // 2)
                            ydst = nxt_i[mb] if nxt_i is not None else c3[h][:, r, :]
                            e = P2Y_ENG[li][mb]
                            if e == "v":
                                nc.vector.tensor_copy(ydst[:, :], ps[:, :])
                            else:
                                nc.scalar.copy(ydst[:, :], ps[:, :])
                            eng(SUB_ENG[li][mb]).tensor_sub(
                                w[h][:, r, :], carrier[mb][:, :], ydst[:, :]
                            )

                        _conv_pass(nc, ksb, at[img], segs, pspool, evac_y)

                        half = P * NB // 2
                        if not last:
                            for h in range(2):
                                nc.sync.dma_start(
                                    out[
                                        img, li, h * half : (h + 1) * half
                                    ].rearrange("(b p) w -> p b w", p=P),
                                    w_halves[h][:],
                                )
                        else:
                            # last level: finer-granularity stores; c3 rides
                            # a second DGE queue in parallel with w3.
                            g = L3_STORE_BLOCKS
                            for h in range(2):
                                for q in range(NB // 2 // g):
                                    qi = NB // 2 // g * h + q
                                    dst = slice(qi * P * g, (qi + 1) * P * g)
                                    src = w_halves[h][:, q * g : (q + 1) * g, :]
                                    c3s = c3_halves[h][:, q * g : (q + 1) * g, :]
                                    if g > 1:
                                        dst_ap_w = out[img, li, dst].rearrange(
                                            "(b p) w -> p b w", p=P
                                        )
                                        dst_ap_c = out[img, 3, dst].rearrange(
                                            "(b p) w -> p b w", p=P
                                        )
                                    else:
                                        dst_ap_w = out[img, li, dst]
                                        dst_ap_c = out[img, 3, dst]
                                        src = w_halves[h][:, q, :]
                                        c3s = c3_halves[h][:, q, :]
                                    nc.sync.dma_start(dst_ap_w, src)
                                    nc.sync.dma_start(dst_ap_c, c3s)
                    cur = nxt
                    if li == 1 and _rep + 1 < repeat:
                        pending_cur = issue_loads()
    nc.compile()
    return nc


def _kall_array() -> np.ndarray:
    consts = _const_arrays()
    return np.ascontiguousarray(
        np.concatenate([consts[nm] for nm in consts], axis=1), dtype=np.float16
    )


def _in_maps(x: np.ndarray) -> list[dict]:
    x16 = np.ascontiguousarray(x, dtype=np.float16)
    assert x16.shape == (BPC * NCORES, L, L), x16.shape
    kall = _kall_array()
    return [
        {"x": np.ascontiguousarray(x16[c * BPC : (c + 1) * BPC]), "kall": kall}
        for c in range(NCORES)
    ]


_NC_CACHE = None


def _get_nc():
    global _NC_CACHE
    if _NC_CACHE is None:
        _NC_CACHE = _build_nc()
    return _NC_CACHE


def _run(x: np.ndarray, **spmd_kwargs):
    nc = _get_nc()
    res = run_bass_kernel_spmd(
        nc, _in_maps(x), core_ids=list(range(NCORES)), **spmd_kwargs
    )
    full = np.concatenate(
        [res.results[c]["out"] for c in range(NCORES)], axis=0
    ).astype(np.float32)
    return full, res


def kernel(x: np.ndarray) -> np.ndarray:
    full, _ = _run(x)
    return full


# revision 6
# speedup vs baseline: 1.1652x; 1.1652x over previous
"""B3-spline undecimated wavelet transform (3 levels, reflect BC) on 8 trn2 cores.

Strategy
--------
Pure data parallel: 16 images -> 2 images per core.

Per level the separable 5-tap conv y = K_d @ Y @ K_d^T is computed as two
TensorEngine passes that each convolve along the *partition* axis and
transpose "for free":

    pass1:  AT = (K @ Y)^T      matmul(lhsT=Y_block, rhs=K^T_block)
    pass2:  Ynew = (K @ AT)^T   matmul(lhsT=AT_block, rhs=K^T_block)

K_d is banded (halfwidth 2d <= 8), so for each 128-row contraction block cb
only a narrow output window [cb*128-hw, cb*128+128+hw) is nonzero; each
window is issued as 1-2 matmuls (split at the 512-col PSUM bank boundary)
accumulating into a [128,1024] PSUM tile via the per-element has_written
bits.

HBM traffic is minimized: x is staged fp16 by the host (the conv weights
are dyadic rationals and exact in fp16, accumulation is fp32 in PSUM), and
all 4 output planes are stored fp16 and widened to fp32 on the host, so
per-core traffic is 4.2 MB in + 16.8 MB out (vs 8.4 + 33.6 for fp32).

The two images per core are interleaved pass-by-pass so one image's matmul
stream covers the other's PSUM-evacuation latency. PSUM evacuation (the
only PSUM readers are DVE and ACT) is split per-tile between the two, with
the wavelet subtraction w = Y_prev - Y running fp16 SBUF->SBUF (2x DVE
mode) and a few subs offloaded to GPSIMD.
"""

import sys

if "/opt/trn_rl_repo" not in sys.path:
    sys.path.insert(0, "/opt/trn_rl_repo")

import numpy as np

import concourse.bass as bass
import concourse.mybir as mybir
import concourse.tile as tile
from concourse import bacc
from concourse.bass_utils import run_bass_kernel_spmd

P = 128
L = 1024
NB = L // P            # 8 blocks per axis
BPC = 2                # images per core
NCORES = 8
LEVELS = (1, 2, 4)     # dilation per level
F32 = mybir.dt.float32
F16 = mybir.dt.float16
W5 = (1.0 / 16, 1.0 / 4, 3.0 / 8, 1.0 / 4, 1.0 / 16)

# Per-tile engine assignment (8 chars per level, one per mb tile):
#   'v' = DVE (vector), 's' = ACT (scalar), 'g' = GPSIMD (subs only).
P1_ENG = ("vsvssvss", "vsvssvss", "vsvssvss")   # pass1 A-evac (PSUM copy)
P2Y_ENG = ("svsvsvsv", "svsvsvsv", "svsvsvsv")  # pass2 Y-copy (PSUM copy)
SUB_ENG = ("vggvggvg", "vggvggvg", "gvggvggg")  # w-sub (fp16 SBUF->SBUF)
L3_STORE_BLOCKS = 4  # h-blocks per last-level store DMA (1, 2, or 4)


def _conv_matrix(d: int) -> np.ndarray:
    """K such that (K @ x) == dilated reflect-padded 5-tap conv along axis 0."""
    eye = np.eye(L, dtype=np.float64)
    xp = np.pad(eye, ((2 * d, 2 * d), (0, 0)), mode="reflect")
    K = np.zeros((L, L), dtype=np.float64)
    for k in range(5):
        K += W5[k] * xp[k * d : k * d + L]
    return K.astype(np.float32)


def _const_arrays() -> dict[str, np.ndarray]:
    """fp16 K^T blocks per level: interior Toeplitz block + the two edge blocks."""
    consts = {}
    for li, d in enumerate(LEVELS):
        hw = 2 * d
        KT = _conv_matrix(d).T  # KT[i, n] = K[n, i]
        kint = KT[P : 2 * P, P - hw : 2 * P + hw]
        k0 = KT[0:P, 0 : P + hw]
        k7 = KT[7 * P : 8 * P, 7 * P - hw : 8 * P]
        for nm, a in ((f"kint{li}", kint), (f"k0{li}", k0), (f"k7{li}", k7)):
            a16 = np.ascontiguousarray(a, dtype=np.float16)
            assert np.array_equal(a16.astype(np.float32), a.astype(np.float32))
            consts[nm] = a16
    return consts


def _windows(li: int, cb: int):
    """Nonzero output-column segments for contraction block cb, split at the
    PSUM bank boundary. Returns [(c0, c1, const_name, rhs_col_offset)]."""
    hw = 2 * LEVELS[li]
    if cb == 0:
        c0, c1, nm, base = 0, P + hw, f"k0{li}", 0
    elif cb == NB - 1:
        c0, c1, nm, base = 7 * P - hw, L, f"k7{li}", 7 * P - hw
    else:
        c0, c1, nm, base = cb * P - hw, cb * P + P + hw, f"kint{li}", cb * P - hw
    segs = [(c0, 512), (512, c1)] if c0 < 512 < c1 else [(c0, c1)]
    return [(a, b, nm, a - base) for a, b in segs]


def _mm_list(li: int):
    """Ordered matmul segments for one PSUM tile with per-bank start/stop."""
    segs = []
    for cb in range(NB):
        for a, b, nm, off in _windows(li, cb):
            segs.append([cb, a, b, nm, off, False, False])
    first, last = {}, {}
    for i, s in enumerate(segs):
        bank = s[1] // 512
        first.setdefault(bank, i)
        last[bank] = i
    for i in first.values():
        segs[i][5] = True  # start: clears the bank's has_written bits
    for i in last.values():
        segs[i][6] = True  # stop: closes the accumulation group
    return [tuple(s) for s in segs]


def _conv_pass(nc, ksb, src_tiles, segs, pspool, consume):
    """One transposing conv pass: 8 src tiles [P, L] fp16 -> 8 PSUM tiles [P, L]."""
    for mb in range(NB):
        ps = pspool.tile([P, L], F32, tag="ps", name="ps")
        for cb, a, b, nm, off, st, sp in segs:
            nc.tensor.matmul(
                ps[:, a:b],
                src_tiles[cb][:, mb * P : (mb + 1) * P],
                ksb[nm][:, off : off + (b - a)],
                start=st,
                stop=sp,
            )
        consume(mb, ps)


def _build_nc(repeat: int = 1):
    consts = _const_arrays()
    nc = bacc.Bacc(
        "TRN2",
        target_bir_lowering=False,
        debug=False,
        num_devices=NCORES,
    )
    x_in = nc.dram_tensor("x", [BPC, L, L], F16, kind="ExternalInput")
    out = nc.dram_tensor("out", [BPC, 4, L, L], F16, kind="ExternalOutput")
    knames = list(consts)
    kwidths = [consts[nm].shape[1] for nm in knames]
    koffs = dict(zip(knames, np.cumsum([0] + kwidths[:-1]).tolist()))
    ktotal = int(sum(kwidths))
    kall = nc.dram_tensor("kall", [P, ktotal], F16, kind="ExternalInput")

    def eng(c):
        return {"v": nc.vector, "s": nc.scalar, "g": nc.gpsimd}[c]

    with tile.TileContext(nc) as tc:
        with (
            tc.tile_pool(name="consts", bufs=1) as cpool,
            tc.tile_pool(name="f16", bufs=3 * NB) as fpool,
            tc.tile_pool(name="at", bufs=2 * NB + 4) as apool,
            tc.tile_pool(name="wout", bufs=8) as wpool,
            tc.tile_pool(name="ps", bufs=4, space="PSUM") as pspool,
        ):
            kall_sb = cpool.tile([P, ktotal], F16, name="kall_sb")
            ksb = {
                nm: kall_sb[:, koffs[nm] : koffs[nm] + consts[nm].shape[1]]
                for nm in knames
            }

            kall_loaded = False

            def issue_loads():
                # fp16 x: per-block loads straight into matmul-input tiles,
                # split across the Pool and SP DGE queues. The first block's
                # DMA goes ahead of the (FIFO) const load.
                nonlocal kall_loaded
                cur = {}
                for img in range(BPC):
                    q = nc.gpsimd if img == 0 else nc.sync
                    cur[img] = []
                    for b in range(NB):
                        ct = fpool.tile([P, L], F16, tag="cur", name="cur")
                        q.dma_start(ct[:], x_in[img, b * P : (b + 1) * P])
                        cur[img].append(ct)
                        if not kall_loaded:
                            nc.scalar.dma_start(kall_sb[:], kall[:, :])
                            kall_loaded = True
                return cur

            pending_cur = issue_loads()
            for _rep in range(repeat):
                cur = pending_cur

                for li in range(len(LEVELS)):
                    segs = _mm_list(li)
                    last = li == len(LEVELS) - 1

                    # pass 1: AT = (K @ Y)^T, evacuated to fp16 per block.
                    # Image-interleaved: img1's matmuls cover img0's evacs.
                    at = {}
                    for img in range(BPC):
                        at[img] = [
                            apool.tile([P, L], F16, tag="at", name="at")
                            for _ in range(NB)
                        ]

                        def evac_at(mb, ps, at_i=at[img]):
                            e = P1_ENG[li][mb]
                            if e == "v":
                                nc.vector.tensor_copy(at_i[mb][:, :], ps[:, :])
                            else:
                                nc.scalar.copy(at_i[mb][:, :], ps[:, :])

                        _conv_pass(nc, ksb, cur[img], segs, pspool, evac_at)

                    # pass 2: Ynew = (K @ AT)^T. ACT/DVE copy Ynew fp16 (next
                    # level's input, or c3); the wavelet sub w = Y - Ynew runs
                    # fp16 SBUF->SBUF on DVE (2x mode) or GPSIMD. Output
                    # staged in half-image tiles so stores start early.
                    nxt = {}
                    for img in range(BPC):
                        w_halves = [
                            wpool.tile([P, NB // 2, L], F16, tag="w", name="w_sb")
                            for _ in range(2)
                        ]
                        c3_halves = (
                            [
                                wpool.tile(
                                    [P, NB // 2, L], F16, tag="w", name="c3_sb"
                                )
                                for _ in range(2)
                            ]
                            if last
                            else None
                        )
                        nxt[img] = (
                            None
                            if last
                            else [
                                fpool.tile([P, L], F16, tag="cur", name="nxt")
                                for _ in range(NB)
                            ]
                        )

                        def evac_y(
                            mb,
                            ps,
                            w=w_halves,
                            nxt_i=nxt[img],
                            c3=c3_halves,
                            carrier=cur[img],
                        ):
                            h, r = divmod(mb, NB // 2)
                            ydst = nxt_i[mb] if nxt_i is not None else c3[h][:, r, :]
                            e = P2Y_ENG[li][mb]
                            if e == "v":
                                nc.vector.tensor_copy(ydst[:, :], ps[:, :])
                            else:
                                nc.scalar.copy(ydst[:, :], ps[:, :])
                            eng(SUB_ENG[li][mb]).tensor_sub(
                                w[h][:, r, :], carrier[mb][:, :], ydst[:, :]
                            )

                        _conv_pass(nc, ksb, at[img], segs, pspool, evac_y)

                        half = P * NB // 2
                        if not last:
                            for h in range(2):
                                nc.sync.dma_start(
                                    out[
                                        img, li, h * half : (h + 1) * half
                                    ].rearrange("(b p) w -> p b w", p=P),
                                    w_halves[h][:],
                                )
                        else:
                            # last level: finer-granularity stores; c3 rides
                            # a second DGE queue in parallel with w3.
                            g = L3_STORE_BLOCKS
                            for h in range(2):
                                for q in range(NB // 2 // g):
                                    qi = NB // 2 // g * h + q
                                    dst = slice(qi * P * g, (qi + 1) * P * g)
                                    src = w_halves[h][:, q * g : (q + 1) * g, :]
                                    c3s = c3_halves[h][:, q * g : (q + 1) * g, :]
                                    if g > 1:
                                        dst_ap_w = out[img, li, dst].rearrange(
                                            "(b p) w -> p b w", p=P
                                        )
                                        dst_ap_c = out[img, 3, dst].rearrange(
                                            "(b p) w -> p b w", p=P
                                        )
                                    else:
                                        dst_ap_w = out[img, li, dst]
                                        dst_ap_c = out[img, 3, dst]
                                        src = w_halves[h][:, q, :]
                                        c3s = c3_halves[h][:, q, :]
                                    nc.sync.dma_start(dst_ap_w, src)
                                    nc.sync.dma_start(dst_ap_c, c3s)
                    cur = nxt
                    if li == 1 and _rep + 1 < repeat:
                        pending_cur = issue_loads()
    nc.compile()
    return nc


def _kall_array() -> np.ndarray:
    consts = _const_arrays()
    return np.ascontiguousarray(
        np.concatenate([consts[nm] for nm in consts], axis=1), dtype=np.float16
    )


def _in_maps(x: np.ndarray) -> list[dict]:
    x16 = np.ascontiguousarray(x, dtype=np.float16)
    assert x16.shape == (BPC * NCORES, L, L), x16.shape
    kall = _kall_array()
    return [
        {"x": np.ascontiguousarray(x16[c * BPC : (c + 1) * BPC]), "kall": kall}
        for c in range(NCORES)
    ]


_NC_CACHE = None


def _get_nc():
    global _NC_CACHE
    if _NC_CACHE is None:
        _NC_CACHE = _build_nc()
    return _NC_CACHE


def _run(x: np.ndarray, **spmd_kwargs):
    nc = _get_nc()
    res = run_bass_kernel_spmd(
        nc, _in_maps(x), core_ids=list(range(NCORES)), **spmd_kwargs
    )
    full = np.concatenate(
        [res.results[c]["out"] for c in range(NCORES)], axis=0
    ).astype(np.float32)
    return full, res


def kernel(x: np.ndarray) -> np.ndarray:
    full, _ = _run(x)
    return full


# revision 10
# speedup vs baseline: 1.2814x; 1.0997x over previous
"""B3-spline undecimated wavelet transform (3 levels, reflect BC) on 8 trn2 cores.

Strategy
--------
Pure data parallel: 16 images -> 2 images per core.

Per level the separable 5-tap conv y = K_d @ Y @ K_d^T is computed as two
TensorEngine passes that each convolve along the *partition* axis and
transpose "for free":

    pass1:  AT = (K @ Y)^T      matmul(lhsT=Y_block, rhs=K^T_block)
    pass2:  Ynew = (K @ AT)^T   matmul(lhsT=AT_block, rhs=K^T_block)

K_d is banded (halfwidth 2d <= 8), so for each 128-row contraction block cb
only a narrow output window [cb*128-hw, cb*128+128+hw) is nonzero; each
window is issued as 1-2 matmuls (split at the 512-col PSUM bank boundary)
accumulating into a [128,1024] PSUM tile via the per-element has_written
bits.

HBM traffic is minimized (the measured store ceiling is ~300 GB/s): x is
staged fp16 by the host in a partition-major layout (one 2 MB DMA per
image); w1 (91.6% of output energy) is stored fp16; w2/w3/c3 are stored
fp8e4m3 (total quantization error ~8e-3 against the 2e-2 budget). All
stores go out in the SBUF-native [p, b, w] layout (8-16KB contiguous per
partition) and the host un-permutes + widens to fp32.

The two images per core are interleaved pass-by-pass so one image's matmul
stream covers the other's PSUM-evacuation latency. PSUM evacuation (only
DVE and ACT can read PSUM) is split per-tile between the two; the wavelet
subtraction w = Y_prev - Y runs SBUF->SBUF on DVE (fp16, 2x mode) or
GPSIMD. The next repeat's x loads are prefetched mid-repeat.
"""

import sys

if "/opt/trn_rl_repo" not in sys.path:
    sys.path.insert(0, "/opt/trn_rl_repo")

import numpy as np

import concourse.bass as bass
import concourse.mybir as mybir
import concourse.tile as tile
from concourse import bacc
from concourse.bass_utils import run_bass_kernel_spmd

P = 128
L = 1024
NB = L // P            # 8 blocks per axis
NH = NB // 2           # blocks per half-image store
BPC = 2                # images per core
NCORES = 8
LEVELS = (1, 2, 4)     # dilation per level
F32 = mybir.dt.float32
F16 = mybir.dt.float16
F8 = mybir.dt.float8e4
W5 = (1.0 / 16, 1.0 / 4, 3.0 / 8, 1.0 / 4, 1.0 / 16)

# Per-tile engine assignment (8 chars per level, one per mb tile):
#   'v' = DVE (vector), 's' = ACT (scalar), 'g' = GPSIMD (subs only).
P1_ENG = ("vsvsvsvs", "vsvsvsvs", "vsvsvsvs")   # pass1 A-evac (PSUM copy)
P2Y_ENG = ("svsvsvsv", "svsvsvsv", "ssssssss")  # pass2 Y-copy (PSUM copy)
SUB_ENG = ("vvvvvvvv", "vggggggg", "vggggggg")  # w-sub (SBUF->SBUF)


def _conv_matrix(d: int) -> np.ndarray:
    """K such that (K @ x) == dilated reflect-padded 5-tap conv along axis 0."""
    eye = np.eye(L, dtype=np.float64)
    xp = np.pad(eye, ((2 * d, 2 * d), (0, 0)), mode="reflect")
    K = np.zeros((L, L), dtype=np.float64)
    for k in range(5):
        K += W5[k] * xp[k * d : k * d + L]
    return K.astype(np.float32)


def _const_arrays() -> dict[str, np.ndarray]:
    """fp16 K^T blocks per level: interior Toeplitz block + the two edge blocks."""
    consts = {}
    for li, d in enumerate(LEVELS):
        hw = 2 * d
        KT = _conv_matrix(d).T  # KT[i, n] = K[n, i]
        kint = KT[P : 2 * P, P - hw : 2 * P + hw]
        k0 = KT[0:P, 0 : P + hw]
        k7 = KT[7 * P : 8 * P, 7 * P - hw : 8 * P]
        for nm, a in ((f"kint{li}", kint), (f"k0{li}", k0), (f"k7{li}", k7)):
            a16 = np.ascontiguousarray(a, dtype=np.float16)
            assert np.array_equal(a16.astype(np.float32), a.astype(np.float32))
            consts[nm] = a16
    return consts


def _windows(li: int, cb: int):
    """Nonzero output-column segments for contraction block cb, split at the
    PSUM bank boundary. Returns [(c0, c1, const_name, rhs_col_offset)]."""
    hw = 2 * LEVELS[li]
    if cb == 0:
        c0, c1, nm, base = 0, P + hw, f"k0{li}", 0
    elif cb == NB - 1:
        c0, c1, nm, base = 7 * P - hw, L, f"k7{li}", 7 * P - hw
    else:
        c0, c1, nm, base = cb * P - hw, cb * P + P + hw, f"kint{li}", cb * P - hw
    segs = [(c0, 512), (512, c1)] if c0 < 512 < c1 else [(c0, c1)]
    return [(a, b, nm, a - base) for a, b in segs]


def _mm_list(li: int):
    """Ordered matmul segments for one PSUM tile with per-bank start/stop."""
    segs = []
    for cb in range(NB):
        for a, b, nm, off in _windows(li, cb):
            segs.append([cb, a, b, nm, off, False, False])
    first, last = {}, {}
    for i, s in enumerate(segs):
        bank = s[1] // 512
        first.setdefault(bank, i)
        last[bank] = i
    for i in first.values():
        segs[i][5] = True  # start: clears the bank's has_written bits
    for i in last.values():
        segs[i][6] = True  # stop: closes the accumulation group
    return [tuple(s) for s in segs]


def _conv_pass(nc, ksb, src_tiles, segs, pspool, consume):
    """One transposing conv pass: 8 src tiles [P, L] fp16 -> 8 PSUM tiles [P, L]."""
    for mb in range(NB):
        ps = pspool.tile([P, L], F32, tag="ps", name="ps")
        for cb, a, b, nm, off, st, sp in segs:
            nc.tensor.matmul(
                ps[:, a:b],
                src_tiles[cb][:, mb * P : (mb + 1) * P],
                ksb[nm][:, off : off + (b - a)],
                start=st,
                stop=sp,
            )
        consume(mb, ps)


def _build_nc(repeat: int = 1):
    consts = _const_arrays()
    nc = bacc.Bacc(
        "TRN2",
        target_bir_lowering=False,
        debug=False,
        num_devices=NCORES,
    )
    # x pre-permuted by the host: x_dev[img, p, b, w] = x[img, b*128+p, w]
    x_in = nc.dram_tensor("x", [BPC, P, NB, L], F16, kind="ExternalInput")
    # outputs in SBUF-native half layout: [img, half, p, b, w];
    # plane row (h*4+b)*128+p. Host un-permutes.
    out_w1 = nc.dram_tensor("w1", [BPC, 2, P, NH, L], F16, kind="ExternalOutput")
    out_r = nc.dram_tensor("r", [3, BPC, 2, P, NH, L], F8, kind="ExternalOutput")
    knames = list(consts)
    kwidths = [consts[nm].shape[1] for nm in knames]
    koffs = dict(zip(knames, np.cumsum([0] + kwidths[:-1]).tolist()))
    ktotal = int(sum(kwidths))
    kall = nc.dram_tensor("kall", [P, ktotal], F16, kind="ExternalInput")

    def eng(c):
        return {"v": nc.vector, "s": nc.scalar, "g": nc.gpsimd}[c]

    with tile.TileContext(nc) as tc:
        with (
            tc.tile_pool(name="consts", bufs=1) as cpool,
            tc.tile_pool(name="xin", bufs=3) as xpool,
            tc.tile_pool(name="nxt", bufs=3 * NB) as fpool,
            tc.tile_pool(name="at", bufs=2 * NB) as apool,
            tc.tile_pool(name="wout", bufs=4) as wpool,
            tc.tile_pool(name="ps", bufs=4, space="PSUM") as pspool,
        ):
            kall_sb = cpool.tile([P, ktotal], F16, name="kall_sb")
            ksb = {
                nm: kall_sb[:, koffs[nm] : koffs[nm] + consts[nm].shape[1]]
                for nm in knames
            }

            kall_loaded = False

            def issue_loads():
                # one whole-image DMA per image (16KB contiguous/partition),
                # on the Pool DGE queue; the const load rides after img0's.
                nonlocal kall_loaded
                cur = {}
                for img in range(BPC):
                    xt = xpool.tile([P, NB, L], F16, tag="x", name="x_sb")
                    nc.gpsimd.dma_start(xt[:], x_in[img])
                    cur[img] = [xt[:, b, :] for b in range(NB)]
                    if not kall_loaded:
                        nc.scalar.dma_start(kall_sb[:], kall[:, :])
                        kall_loaded = True
                return cur

            pending_cur = issue_loads()
            for _rep in range(repeat):
                cur = pending_cur

                for li in range(len(LEVELS)):
                    segs = _mm_list(li)
                    last = li == len(LEVELS) - 1

                    # pass 1: AT = (K @ Y)^T, evacuated to fp16 per block.
                    # Image-interleaved: img1's matmuls cover img0's evacs.
                    at = {}
                    for img in range(BPC):
                        at[img] = [
                            apool.tile([P, L], F16, tag="at", name="at")
                            for _ in range(NB)
                        ]

                        def evac_at(mb, ps, at_i=at[img]):
                            e = P1_ENG[li][mb]
                            if e == "v":
                                nc.vector.tensor_copy(at_i[mb][:, :], ps[:, :])
                            else:
                                nc.scalar.copy(at_i[mb][:, :], ps[:, :])

                        _conv_pass(nc, ksb, cur[img], segs, pspool, evac_at)

                    # pass 2: Ynew = (K @ AT)^T. ACT/DVE copy Ynew (fp16 next
                    # level input, or fp8 c3); the wavelet sub runs
                    # SBUF->SBUF on DVE or GPSIMD. w1 fp16, w2/w3 fp8.
                    nxt = {}
                    for img in range(BPC):
                        wdt = F16 if li == 0 else F8
                        w_halves = [
                            wpool.tile(
                                [P, NH, L],
                                wdt,
                                tag="w16" if li == 0 else "w8",
                                bufs=4 if li == 0 else 6,
                                name="w_sb",
                            )
                            for _ in range(2)
                        ]
                        c3_halves = (
                            [
                                wpool.tile(
                                    [P, NH, L], F8, tag="w8", bufs=6, name="c3_sb"
                                )
                                for _ in range(2)
                            ]
                            if last
                            else None
                        )
                        nxt[img] = (
                            None
                            if last
                            else [
                                fpool.tile([P, L], F16, tag="cur", name="nxt")
                                for _ in range(NB)
                            ]
                        )

                        def evac_y(
                            mb,
                            ps,
                            w=w_halves,
                            nxt_i=nxt[img],
                            c3=c3_halves,
                            carrier=cur[img],
                        ):
                            h, r = divmod(mb, NH)
                            ydst = nxt_i[mb] if nxt_i is not None else c3[h][:, r, :]
                            e = P2Y_ENG[li][mb]
                            if e == "v":
                                nc.vector.tensor_copy(ydst[:, :], ps[:, :])
                            else:
                                nc.scalar.copy(ydst[:, :], ps[:, :])
                            eng(SUB_ENG[li][mb]).tensor_sub(
                                w[h][:, r, :], carrier[mb][:, :], ydst[:, :]
                            )

                        _conv_pass(nc, ksb, at[img], segs, pspool, evac_y)

                        for h in range(2):
                            if li == 0:
                                nc.sync.dma_start(
                                    out_w1[img, h], w_halves[h][:]
                                )
                            else:
                                nc.sync.dma_start(
                                    out_r[li - 1, img, h], w_halves[h][:]
                                )
                            if last:
                                nc.sync.dma_start(
                                    out_r[2, img, h], c3_halves[h][:]
                                )
                    cur = nxt
                    if li == 1 and _rep + 1 < repeat:
                        pending_cur = issue_loads()
    nc.compile()
    return nc


def _kall_array() -> np.ndarray:
    consts = _const_arrays()
    return np.ascontiguousarray(
        np.concatenate([consts[nm] for nm in consts], axis=1), dtype=np.float16
    )


def _in_maps(x: np.ndarray) -> list[dict]:
    x16 = np.asarray(x, dtype=np.float16)
    assert x16.shape == (BPC * NCORES, L, L), x16.shape
    # [n, b*128+p, w] -> [n, p, b, w]
    xp = np.ascontiguousarray(
        x16.reshape(BPC * NCORES, NB, P, L).transpose(0, 2, 1, 3)
    )
    kall = _kall_array()
    return [
        {"x": xp[c * BPC : (c + 1) * BPC], "kall": kall}
        for c in range(NCORES)
    ]


def _assemble(w1_parts: list[np.ndarray], r_parts: list[np.ndarray]) -> np.ndarray:
    """Un-permute device outputs into the reference (B, 4, L, L) fp32 layout."""
    w1 = np.concatenate(w1_parts, axis=0)  # [B, 2, P, NH, L] fp16
    full = np.empty((w1.shape[0], 4, L, L), np.float32)
    r = np.concatenate(r_parts, axis=1).astype(np.float32)  # [3, B, 2, P, NH, L]
    # [B, h, p, b, w] -> row (h*NH+b)*P+p
    full[:, 0] = (
        w1.astype(np.float32).transpose(0, 1, 3, 2, 4).reshape(-1, L, L)
    )
    for j in range(3):
        full[:, j + 1] = r[j].transpose(0, 1, 3, 2, 4).reshape(-1, L, L)
    return full


_NC_CACHE = None


def _get_nc():
    global _NC_CACHE
    if _NC_CACHE is None:
        _NC_CACHE = _build_nc()
    return _NC_CACHE


def _run(x: np.ndarray, **spmd_kwargs):
    nc = _get_nc()
    res = run_bass_kernel_spmd(
        nc, _in_maps(x), core_ids=list(range(NCORES)), **spmd_kwargs
    )
    full = _assemble(
        [res.results[c]["w1"] for c in range(NCORES)],
        [res.results[c]["r"] for c in range(NCORES)],
    )
    return full, res


def kernel(x: np.ndarray) -> np.ndarray:
    full, _ = _run(x)
    return full


# revision 11
# speedup vs baseline: 1.3030x; 1.0169x over previous
"""B3-spline undecimated wavelet transform (3 levels, reflect BC) on 8 trn2 cores.

Strategy
--------
Pure data parallel: 16 images -> 2 images per core.

Per level the separable 5-tap conv y = K_d @ Y @ K_d^T is computed as two
TensorEngine passes that each convolve along the *partition* axis and
transpose "for free":

    pass1:  AT = (K @ Y)^T      matmul(lhsT=Y_block, rhs=K^T_block)
    pass2:  Ynew = (K @ AT)^T   matmul(lhsT=AT_block, rhs=K^T_block)

K_d is banded (halfwidth 2d <= 8), so for each 128-row contraction block cb
only a narrow output window [cb*128-hw, cb*128+128+hw) is nonzero; each
window is issued as 1-2 matmuls (split at the 512-col PSUM bank boundary)
accumulating into a [128,1024] PSUM tile via the per-element has_written
bits.

HBM traffic is minimized (the measured store ceiling is ~300 GB/s): x is
staged fp16 by the host in a partition-major layout (one 2 MB DMA per
image); w1 (91.6% of output energy) is stored fp16; w2/w3/c3 are stored
fp8e4m3 (total quantization error ~8e-3 against the 2e-2 budget). All
stores go out in the SBUF-native [p, b, w] layout (8-16KB contiguous per
partition) and the host un-permutes + widens to fp32.

The two images per core are interleaved pass-by-pass so one image's matmul
stream covers the other's PSUM-evacuation latency. PSUM evacuation (only
DVE and ACT can read PSUM) is split per-tile between the two; the wavelet
subtraction w = Y_prev - Y runs SBUF->SBUF on DVE (fp16, 2x mode) or
GPSIMD. The next repeat's x loads are prefetched mid-repeat.
"""

import sys

if "/opt/trn_rl_repo" not in sys.path:
    sys.path.insert(0, "/opt/trn_rl_repo")

import numpy as np

import concourse.bass as bass
import concourse.mybir as mybir
import concourse.tile as tile
from concourse import bacc
from concourse.bass_utils import run_bass_kernel_spmd

P = 128
L = 1024
NB = L // P            # 8 blocks per axis
NH = NB // 2           # blocks per half-image store
BPC = 2                # images per core
NCORES = 8
LEVELS = (1, 2, 4)     # dilation per level
F32 = mybir.dt.float32
F16 = mybir.dt.float16
F8 = mybir.dt.float8e4
W5 = (1.0 / 16, 1.0 / 4, 3.0 / 8, 1.0 / 4, 1.0 / 16)

# Per-tile engine assignment (8 chars per level, one per mb tile):
#   'v' = DVE (vector), 's' = ACT (scalar), 'g' = GPSIMD (subs only).
P1_ENG = ("vsvsvsss", "vsvsvsss", "vsvsvsvs")   # pass1 A-evac (PSUM copy)
P2Y_ENG = ("svsvssvs", "svsvssvs", "ssvssvss")  # pass2 Y-copy (PSUM copy)
SUB_ENG = ("vvgvvgvg", "vgvgvggg", "vgvgvggg")  # w-sub (SBUF->SBUF)


def _conv_matrix(d: int) -> np.ndarray:
    """K such that (K @ x) == dilated reflect-padded 5-tap conv along axis 0."""
    eye = np.eye(L, dtype=np.float64)
    xp = np.pad(eye, ((2 * d, 2 * d), (0, 0)), mode="reflect")
    K = np.zeros((L, L), dtype=np.float64)
    for k in range(5):
        K += W5[k] * xp[k * d : k * d + L]
    return K.astype(np.float32)


def _const_arrays() -> dict[str, np.ndarray]:
    """fp16 K^T blocks per level: interior Toeplitz block + the two edge blocks."""
    consts = {}
    for li, d in enumerate(LEVELS):
        hw = 2 * d
        KT = _conv_matrix(d).T  # KT[i, n] = K[n, i]
        kint = KT[P : 2 * P, P - hw : 2 * P + hw]
        k0 = KT[0:P, 0 : P + hw]
        k7 = KT[7 * P : 8 * P, 7 * P - hw : 8 * P]
        for nm, a in ((f"kint{li}", kint), (f"k0{li}", k0), (f"k7{li}", k7)):
            a16 = np.ascontiguousarray(a, dtype=np.float16)
            assert np.array_equal(a16.astype(np.float32), a.astype(np.float32))
            consts[nm] = a16
    return consts


def _windows(li: int, cb: int):
    """Nonzero output-column segments for contraction block cb, split at the
    PSUM bank boundary. Returns [(c0, c1, const_name, rhs_col_offset)]."""
    hw = 2 * LEVELS[li]
    if cb == 0:
        c0, c1, nm, base = 0, P + hw, f"k0{li}", 0
    elif cb == NB - 1:
        c0, c1, nm, base = 7 * P - hw, L, f"k7{li}", 7 * P - hw
    else:
        c0, c1, nm, base = cb * P - hw, cb * P + P + hw, f"kint{li}", cb * P - hw
    segs = [(c0, 512), (512, c1)] if c0 < 512 < c1 else [(c0, c1)]
    return [(a, b, nm, a - base) for a, b in segs]


def _mm_list(li: int):
    """Ordered matmul segments for one PSUM tile with per-bank start/stop."""
    segs = []
    for cb in range(NB):
        for a, b, nm, off in _windows(li, cb):
            segs.append([cb, a, b, nm, off, False, False])
    first, last = {}, {}
    for i, s in enumerate(segs):
        bank = s[1] // 512
        first.setdefault(bank, i)
        last[bank] = i
    for i in first.values():
        segs[i][5] = True  # start: clears the bank's has_written bits
    for i in last.values():
        segs[i][6] = True  # stop: closes the accumulation group
    return [tuple(s) for s in segs]


def _conv_pass(nc, ksb, src_tiles, segs, pspool, consume):
    """One transposing conv pass: 8 src tiles [P, L] fp16 -> 8 PSUM tiles [P, L]."""
    for mb in range(NB):
        ps = pspool.tile([P, L], F32, tag="ps", name="ps")
        for cb, a, b, nm, off, st, sp in segs:
            nc.tensor.matmul(
                ps[:, a:b],
                src_tiles[cb][:, mb * P : (mb + 1) * P],
                ksb[nm][:, off : off + (b - a)],
                start=st,
                stop=sp,
            )
        consume(mb, ps)


def _build_nc(repeat: int = 1):
    consts = _const_arrays()
    nc = bacc.Bacc(
        "TRN2",
        target_bir_lowering=False,
        debug=False,
        num_devices=NCORES,
    )
    # x pre-permuted by the host: x_dev[img, p, b, w] = x[img, b*128+p, w]
    x_in = nc.dram_tensor("x", [BPC, P, NB, L], F16, kind="ExternalInput")
    # outputs in SBUF-native half layout: [img, half, p, b, w];
    # plane row (h*4+b)*128+p. Host un-permutes.
    out_w1 = nc.dram_tensor("w1", [BPC, 2, P, NH, L], F16, kind="ExternalOutput")
    out_r = nc.dram_tensor("r", [3, BPC, 2, P, NH, L], F8, kind="ExternalOutput")
    knames = list(consts)
    kwidths = [consts[nm].shape[1] for nm in knames]
    koffs = dict(zip(knames, np.cumsum([0] + kwidths[:-1]).tolist()))
    ktotal = int(sum(kwidths))
    kall = nc.dram_tensor("kall", [P, ktotal], F16, kind="ExternalInput")

    def eng(c):
        return {"v": nc.vector, "s": nc.scalar, "g": nc.gpsimd}[c]

    with tile.TileContext(nc) as tc:
        with (
            tc.tile_pool(name="consts", bufs=1) as cpool,
            tc.tile_pool(name="xin", bufs=3) as xpool,
            tc.tile_pool(name="nxt", bufs=3 * NB) as fpool,
            tc.tile_pool(name="at", bufs=2 * NB) as apool,
            tc.tile_pool(name="wout", bufs=4) as wpool,
            tc.tile_pool(name="ps", bufs=4, space="PSUM") as pspool,
        ):
            kall_sb = cpool.tile([P, ktotal], F16, name="kall_sb")
            ksb = {
                nm: kall_sb[:, koffs[nm] : koffs[nm] + consts[nm].shape[1]]
                for nm in knames
            }

            kall_loaded = False

            def issue_loads():
                # one whole-image DMA per image (16KB contiguous/partition),
                # on the Pool DGE queue; the const load rides after img0's.
                nonlocal kall_loaded
                cur = {}
                for img in range(BPC):
                    xt = xpool.tile([P, NB, L], F16, tag="x", name="x_sb")
                    nc.gpsimd.dma_start(xt[:], x_in[img])
                    cur[img] = [xt[:, b, :] for b in range(NB)]
                    if not kall_loaded:
                        nc.scalar.dma_start(kall_sb[:], kall[:, :])
                        kall_loaded = True
                return cur

            pending_cur = issue_loads()
            for _rep in range(repeat):
                cur = pending_cur

                for li in range(len(LEVELS)):
                    segs = _mm_list(li)
                    last = li == len(LEVELS) - 1

                    # pass 1: AT = (K @ Y)^T, evacuated to fp16 per block.
                    # Image-interleaved: img1's matmuls cover img0's evacs.
                    at = {}
                    for img in range(BPC):
                        at[img] = [
                            apool.tile([P, L], F16, tag="at", name="at")
                            for _ in range(NB)
                        ]

                        def evac_at(mb, ps, at_i=at[img]):
                            e = P1_ENG[li][mb]
                            if e == "v":
                                nc.vector.tensor_copy(at_i[mb][:, :], ps[:, :])
                            else:
                                nc.scalar.copy(at_i[mb][:, :], ps[:, :])

                        _conv_pass(nc, ksb, cur[img], segs, pspool, evac_at)

                    # pass 2: Ynew = (K @ AT)^T. ACT/DVE copy Ynew (fp16 next
                    # level input, or fp8 c3); the wavelet sub runs
                    # SBUF->SBUF on DVE or GPSIMD. w1 fp16, w2/w3 fp8.
                    nxt = {}
                    for img in range(BPC):
                        wdt = F16 if li == 0 else F8
                        w_halves = [
                            wpool.tile(
                                [P, NH, L],
                                wdt,
                                tag="w16" if li == 0 else "w8",
                                bufs=4 if li == 0 else 6,
                                name="w_sb",
                            )
                            for _ in range(2)
                        ]
                        c3_halves = (
                            [
                                wpool.tile(
                                    [P, NH, L], F8, tag="w8", bufs=6, name="c3_sb"
                                )
                                for _ in range(2)
                            ]
                            if last
                            else None
                        )
                        nxt[img] = (
                            None
                            if last
                            else [
                                fpool.tile([P, L], F16, tag="cur", name="nxt")
                                for _ in range(NB)
                            ]
                        )

                        def evac_y(
                            mb,
                            ps,
                            w=w_halves,
                            nxt_i=nxt[img],
                            c3=c3_halves,
                            carrier=cur[img],
                        ):
                            h, r = divmod(mb, NH)
                            ydst = nxt_i[mb] if nxt_i is not None else c3[h][:, r, :]
                            e = P2Y_ENG[li][mb]
                            if e == "v":
                                nc.vector.tensor_copy(ydst[:, :], ps[:, :])
                            else:
                                nc.scalar.copy(ydst[:, :], ps[:, :])
                            eng(SUB_ENG[li][mb]).tensor_sub(
                                w[h][:, r, :], carrier[mb][:, :], ydst[:, :]
                            )

                        _conv_pass(nc, ksb, at[img], segs, pspool, evac_y)

                        for h in range(2):
                            if li == 0:
                                nc.sync.dma_start(
                                    out_w1[img, h], w_halves[h][:]
                                )
                            else:
                                nc.sync.dma_start(
                                    out_r[li - 1, img, h], w_halves[h][:]
                                )
                            if last:
                                nc.sync.dma_start(
                                    out_r[2, img, h], c3_halves[h][:]
                                )
                    cur = nxt
                    if li == 1 and _rep + 1 < repeat:
                        pending_cur = issue_loads()
    nc.compile()
    return nc


def _kall_array() -> np.ndarray:
    consts = _const_arrays()
    return np.ascontiguousarray(
        np.concatenate([consts[nm] for nm in consts], axis=1), dtype=np.float16
    )


def _in_maps(x: np.ndarray) -> list[dict]:
    x16 = np.asarray(x, dtype=np.float16)
    assert x16.shape == (BPC * NCORES, L, L), x16.shape
    # [n, b*128+p, w] -> [n, p, b, w]
    xp = np.ascontiguousarray(
        x16.reshape(BPC * NCORES, NB, P, L).transpose(0, 2, 1, 3)
    )
    kall = _kall_array()
    return [
        {"x": xp[c * BPC : (c + 1) * BPC], "kall": kall}
        for c in range(NCORES)
    ]


def _assemble(w1_parts: list[np.ndarray], r_parts: list[np.ndarray]) -> np.ndarray:
    """Un-permute device outputs into the reference (B, 4, L, L) fp32 layout."""
    w1 = np.concatenate(w1_parts, axis=0)  # [B, 2, P, NH, L] fp16
    full = np.empty((w1.shape[0], 4, L, L), np.float32)
    r = np.concatenate(r_parts, axis=1).astype(np.float32)  # [3, B, 2, P, NH, L]
    # [B, h, p, b, w] -> row (h*NH+b)*P+p
    full[:, 0] = (
        w1.astype(np.float32).transpose(0, 1, 3, 2, 4).reshape(-1, L, L)
    )
    for j in range(3):
        full[:, j + 1] = r[j].transpose(0, 1, 3, 2, 4).reshape(-1, L, L)
    return full


_NC_CACHE = None


def _get_nc():
    global _NC_CACHE
    if _NC_CACHE is None:
        _NC_CACHE = _build_nc()
    return _NC_CACHE


def _run(x: np.ndarray, **spmd_kwargs):
    nc = _get_nc()
    res = run_bass_kernel_spmd(
        nc, _in_maps(x), core_ids=list(range(NCORES)), **spmd_kwargs
    )
    full = _assemble(
        [res.results[c]["w1"] for c in range(NCORES)],
        [res.results[c]["r"] for c in range(NCORES)],
    )
    return full, res


def kernel(x: np.ndarray) -> np.ndarray:
    full, _ = _run(x)
    return full
